# revision 4
# baseline (speedup 1.0000x reference)
"""Trainium2 Bass kernel v4: core-level dedup gather + multi-bank matmuls.

out[n] = sum_l softmax(w1+w2)[n,l] * (table[neigh[n,l]] . table[cand[n]])
         reshaped (32, 128)

v4 over v3/v2: each core's 512 pairs split into 4 cells of 128 (PSUM bank
each). A distinct table row referenced by several cells is gathered ONCE:
rows are grouped into sections by their referencing cell-set (15 classes x
lo/hi int16 range); a block in a class-{i,j} section feeds matmuls into
banks i and j (per-bank P slices). Total matmul count is unchanged
(sum of per-cell distinct rows) while gathered rows drop ~35% -> less Q7
descriptor-generation (the serial bottleneck) and less gather DMA.
Everything bf16 with softmax weights folded into P on host; candidates
pre-gathered on host in f32.
"""
import numpy as np

N, L, K, R = 4096, 256, 256, 50000
NCORES = 8
NPC = N // NCORES            # 512 pairs per core
M = 128                      # pairs per cell / bank
GROUPS = NPC // M            # 4 cells per core
HIBASE = 1 << 15
OP_BLOCKS = 8                # descs/op = OPB*128/16+1 = 65: fits the
NQ = 4                       # per-queue SWDGE ring at num_swdge_queues=4
GBUFS = 14
BPC = 8

# section order: (cell-set bitmask, hi). Multi-bank classes first, then the
# big singleton sections so each bank's PSUM chain closes staggered before
# the end (final dot for bank g overlaps the remaining banks' matmuls).
CLASSES = [15, 14, 13, 11, 7, 12, 10, 9, 6, 5, 3, 1, 2, 4, 8]
SECTIONS = [(bits, hi) for bits in CLASSES for hi in (False, True)]


def _banks(bits):
    return [g for g in range(GROUPS) if bits & (1 << g)]


def _softmax_weights(w1, w2, lengths):
    mask = np.arange(L)[None, :] < lengths[:, None]
    lw = (w1 + w2).astype(np.float64)
    lw[~mask] = -np.inf
    lw -= lw.max(axis=1, keepdims=True)
    e = np.exp(lw)
    return e / e.sum(axis=1, keepdims=True)


def _plan(lengths, lowcnt):
    ncells = NCORES * GROUPS
    order = np.argsort(-lengths, kind="stable")
    cells = [[] for _ in range(ncells)]
    low = np.zeros(ncells)
    high = np.zeros(ncells)
    cnt = np.zeros(ncells, dtype=np.int64)
    for n in order:
        lc, hc = float(lowcnt[n]), float(lengths[n] - lowcnt[n])
        cost = np.maximum(low + lc, high + hc * (HIBASE / (R - HIBASE)))
        cost[cnt >= M] = np.inf
        cell = int(np.argmin(cost))
        cells[cell].append(int(n))
        low[cell] += lc
        high[cell] += hc
        cnt[cell] += 1
    return cells


def _core_sections(cells, lengths, neigh_idx, weights, core):
    """Per core: slot arrays and per-(class,hi) deduped row sets.

    Returns:
      sec: dict (bits, hi) -> (uidx sorted array, dict keyed nothing) plus
           per-slot mapping arrays for the P fill.
    """
    idx_a, g_a, j_a, w_a = [], [], [], []
    for g in range(GROUPS):
        for j, n in enumerate(cells[core * GROUPS + g]):
            ln = int(lengths[n])
            idx_a.append(neigh_idx[n, :ln].astype(np.int64))
            g_a.append(np.full(ln, g, dtype=np.int64))
            j_a.append(np.full(ln, j, dtype=np.int64))
            w_a.append(weights[n, :ln])
    idx_a = np.concatenate(idx_a)
    g_a = np.concatenate(g_a)
    j_a = np.concatenate(j_a)
    w_a = np.concatenate(w_a)

    uidx, inv = np.unique(idx_a, return_inverse=True)
    cellmask = np.zeros((len(uidx), GROUPS), dtype=bool)
    cellmask[inv, g_a] = True
    bits_of_row = cellmask @ (1 << np.arange(GROUPS))          # [nrows]
    hi_of_row = uidx >= HIBASE

    # per section: row ids (positions into uidx) in sorted-idx order
    secrows = {}
    rowpos = np.zeros(len(uidx), dtype=np.int64)   # position within section
    for bits in CLASSES:
        for hi in (False, True):
            sel = np.nonzero((bits_of_row == bits) & (hi_of_row == hi))[0]
            secrows[(bits, hi)] = sel
            rowpos[sel] = np.arange(len(sel))
    return (idx_a, g_a, j_a, w_a, uidx, inv, bits_of_row, hi_of_row,
            rowpos, secrows)


def _build_core_arrays(coresec, NBS, cand_map, core):
    (idx_a, g_a, j_a, w_a, uidx, inv, bits_of_row, hi_of_row,
     rowpos, secrows) = coresec

    NB = sum(NBS.values())
    secoff = {}
    pbase = {}
    off = 0
    poff = 0
    for s in SECTIONS:
        secoff[s] = off
        pbase[s] = poff
        off += NBS[s]
        poff += NBS[s] * len(_banks(s[0])) * M
    PCOLS_TOT = poff

    idx16_s = np.zeros((128, NB * BPC), dtype=np.int16)
    P_s = np.zeros((128, PCOLS_TOT), dtype=np.float32)

    # P fill: per slot -> its dedup row -> section/pos/block/slice
    row = inv                                  # dedup row id per slot
    bits = bits_of_row[row]
    hi = hi_of_row[row]
    pos = rowpos[row]
    ks = np.array([bin(b).count("1") for b in range(16)])
    k_arr = ks[bits]
    # rank of cell g within the class bitmask
    rank = np.zeros(len(row), dtype=np.int64)
    for g in range(GROUPS):
        m = g_a == g
        rank[m] = [bin(b & ((1 << g) - 1)).count("1") for b in bits[m]]
    pb = np.array([pbase[(b, h)] for b, h in zip(bits, hi)])
    so = np.array([secoff[(b, h)] for b, h in zip(bits, hi)])
    pcol = pb + (pos // 128) * k_arr * M + rank * M + j_a
    np.add.at(P_s, (pos % 128, pcol), w_a)

    # idx16 streams per section
    for s in SECTIONS:
        bits_s, hi_s = s
        sel = secrows[s]
        base = HIBASE if hi_s else 0
        nblk = NBS[s]
        sidx = np.zeros(nblk * 128, dtype=np.int64)
        sidx[:len(sel)] = uidx[sel] - base
        for o0 in range(0, nblk, OP_BLOCKS):
            nb_op = min(OP_BLOCKS, nblk - o0)
            op_stream = sidx[o0 * 128:(o0 + nb_op) * 128]
            w = nb_op * BPC
            sArr = np.arange(w)
            for pm in range(16):
                vals = op_stream[sArr * 16 + pm].astype(np.int16)
                coff = (secoff[s] + o0) * BPC
                idx16_s[pm::16, coff:coff + w] = vals[None, :]

    cand_s, ns_local = cand_map
    return idx16_s, P_s, cand_s, ns_local


def _build_program(NBS):
    import concourse.mybir as mybir
    import concourse.tile as tile
    from concourse import bacc

    NB = sum(NBS.values())
    secoff = {}
    pbase = {}
    off = 0
    poff = 0
    for s in SECTIONS:
        secoff[s] = off
        pbase[s] = poff
        off += NBS[s]
        poff += NBS[s] * len(_banks(s[0])) * M
    PCOLS_TOT = poff

    nc = bacc.Bacc("TRN2", target_bir_lowering=False, debug=True,
                   num_swdge_queues=NQ)
    f32, i16, bf16 = mybir.dt.float32, mybir.dt.int16, mybir.dt.bfloat16
    table_b = nc.dram_tensor("table_b", [R, K], bf16, kind="ExternalInput")
    idx_d = nc.dram_tensor("idx16_s", [128, NB * BPC], i16,
                           kind="ExternalInput")
    P_d = nc.dram_tensor("P_s", [128, PCOLS_TOT], bf16, kind="ExternalInput")
    cand_d = nc.dram_tensor("cand_s", [128, (NPC // 128) * K], f32,
                            kind="ExternalInput")
    out_d = nc.dram_tensor("out_t", [128, NPC // 128], f32,
                           kind="ExternalOutput")

    # ops: (section, o0, nb_op); mm flags per bank
    ops = []
    for s in SECTIONS:
        for o0 in range(0, NBS[s], OP_BLOCKS):
            ops.append((s, o0, min(OP_BLOCKS, NBS[s] - o0)))
    mm_total = {g: sum(NBS[s] for s in SECTIONS if s[0] & (1 << g))
                for g in range(GROUPS)}
    mm_seen = {g: 0 for g in range(GROUPS)}

    PCOLS = NPC // 128

    def chunked_dma(dst, src, cols, nchunks):
        csz = max(1, (cols + nchunks - 1) // nchunks)
        for c0 in range(0, cols, csz):
            c1 = min(c0 + csz, cols)
            nc.sync.dma_start(out=dst[:, c0:c1], in_=src[:, c0:c1])

    with tile.TileContext(nc) as tc:
        with tc.tile_pool(name="const", bufs=1) as const, \
             tc.tile_pool(name="g", bufs=GBUFS) as gpool, \
             tc.tile_pool(name="fin", bufs=2) as fin, \
             tc.tile_pool(name="psum", bufs=1, space="PSUM") as psum:
            idx_t = const.tile([128, NB * BPC], i16)
            chunked_dma(idx_t, idx_d, NB * BPC, 8)
            P_t = const.tile([128, PCOLS_TOT], bf16)
            chunked_dma(P_t, P_d, PCOLS_TOT, 16)
            cand_t = const.tile([128, PCOLS * K], f32)
            nc.sync.dma_start(out=cand_t[:], in_=cand_d[:])

            agg = [psum.tile([128, K], f32, name=f"agg{i}", tag=f"agg{i}")
                   for i in range(GROUPS)]

            # one count register per distinct op size (instead of a MOVE
            # per gather op in the Pool instruction stream)
            cnt_regs = {}
            for (_, _, nb_op) in ops:
                if nb_op * 128 not in cnt_regs:
                    cnt_regs[nb_op * 128] = nc.gpsimd.to_reg(nb_op * 128)

            q = 0
            for (s, o0, nb_op) in ops:
                bits_s, hi_s = s
                banks = _banks(bits_s)
                k_s = len(banks)
                gb = secoff[s] + o0
                G = gpool.tile([128, OP_BLOCKS * K], bf16, tag="G")
                nc.gpsimd.dma_gather(
                    G[:, :nb_op * K].rearrange("p (b k) -> p b k", b=nb_op),
                    table_b[HIBASE:, :] if hi_s else table_b[:],
                    idx_t[:, gb * BPC:gb * BPC + nb_op * BPC],
                    nb_op * 128,
                    cnt_regs[nb_op * 128],
                    K,
                    queue_num=q % NQ,
                )
                q += 1
                for bl in range(nb_op):
                    pc0 = pbase[s] + (o0 + bl) * k_s * M
                    for r, g in enumerate(banks):
                        mm_seen[g] += 1
                        nc.tensor.matmul(
                            out=agg[g][:],
                            lhsT=P_t[:, pc0 + r * M:pc0 + (r + 1) * M],
                            rhs=G[:, bl * K:(bl + 1) * K],
                            start=(mm_seen[g] == 1),
                            stop=(mm_seen[g] == mm_total[g]),
                            tile_position=(0, 0),
                        )

            out_t = const.tile([128, PCOLS], f32)
            for col in range(PCOLS):
                bank = agg[col]
                scratch = fin.tile([128, K], f32, tag="scratch")
                nc.vector.tensor_mul(
                    out=scratch[:],
                    in0=bank[:],
                    in1=cand_t[:, col * K:(col + 1) * K],
                )
                nc.vector.tensor_reduce(
                    out=out_t[:, col:col + 1],
                    in_=scratch[:],
                    axis=mybir.AxisListType.X,
                    op=mybir.AluOpType.add,
                )
                nc.sync.dma_start(out=out_d[:, col:col + 1],
                                  in_=out_t[:, col:col + 1])
    nc.compile()
    return nc


def kernel(table, w1, w2, cand_idx, neigh_idx, lengths):
    import ml_dtypes
    table = np.ascontiguousarray(table, dtype=np.float32)
    w1 = np.asarray(w1, dtype=np.float32)
    w2 = np.asarray(w2, dtype=np.float32)
    cand_idx = np.asarray(cand_idx, dtype=np.int32)
    neigh_idx = np.asarray(neigh_idx, dtype=np.int32)
    lengths = np.asarray(lengths, dtype=np.int32)

    weights = _softmax_weights(w1, w2, lengths)
    lowcnt = np.array([(neigh_idx[n, :lengths[n]] < HIBASE).sum()
                       for n in range(N)], dtype=np.int64)
    cells = _plan(lengths, lowcnt)

    coresecs = []
    cand_maps = []
    for c in range(NCORES):
        coresecs.append(_core_sections(cells, lengths, neigh_idx, weights, c))
        cand_s = np.zeros((128, (NPC // 128) * K), dtype=np.float32)
        ns_local = np.zeros(NPC, dtype=np.int64)
        for g in range(GROUPS):
            for j, n in enumerate(cells[c * GROUPS + g]):
                i_local = g * M + j
                ns_local[i_local] = n
                cand_s[i_local % 128,
                       (i_local // 128) * K:(i_local // 128 + 1) * K] = \
                    table[cand_idx[n]]
        cand_maps.append((cand_s, ns_local))

    NBS = {}
    for s in SECTIONS:
        NBS[s] = max(1, max((len(cs[9][s]) + 127) // 128 for cs in coresecs))

    in_maps = []
    ns_locals = []
    table_b = table.astype(ml_dtypes.bfloat16)
    for c in range(NCORES):
        idx16_s, P_s, cand_s, ns_local = _build_core_arrays(
            coresecs[c], NBS, cand_maps[c], c)
        in_maps.append({"table_b": table_b,
                        "idx16_s": idx16_s,
                        "P_s": P_s.astype(ml_dtypes.bfloat16),
                        "cand_s": cand_s})
        ns_locals.append(ns_local)

    nc = _build_program(NBS)
    from concourse.bass_utils import run_bass_kernel_spmd
    res = run_bass_kernel_spmd(nc, in_maps, list(range(NCORES)))

    out = np.zeros(N, dtype=np.float32)
    for c in range(NCORES):
        out_t = np.asarray(res.results[c]["out_t"])
        i = np.arange(NPC)
        out[ns_locals[c]] = out_t[i % 128, i // 128]
    return out.reshape(N // 128, 128)
